# revision 22
# baseline (speedup 1.0000x reference)
"""Trainium2 Bass kernel for BitConv2d (act fake-quant int8 + ternary weight 1x1 conv).

Full inputs: x (16, 64, 256, 256) f32, weight (64, 64, 1, 1) f32.
Sharded data-parallel over 8 NeuronCores: 2 batch images per core.
Per-core view of x is [128, 65536]: partition p = n*64 + c, free = h*w.

Math (matches reference):
  xc    = clip(x, -8, 8)
  m     = max(|xc|) over the WHOLE tensor      (global -> AllReduce max)
  s     = max(m, 1e-8) / 127
  u     = round(xc / s)                         (integers in [-127, 127])
  wq    = clip(round(w / s_w), -1, 1) * s_w,    s_w = max(mean|w|, 1e-8) per out-ch
  y     = conv1x1(u * s, wq) = (s * s_w[o]) * (tern @ u)

The integer matmul tern @ u is exact in bf16 x bf16 with fp32 PSUM accumulate.
Rounding uses the hardware float->int conversions, which are
round-half-to-even (= jnp.round) with saturation (= the reference clips).

Modes (env BITCONV_MODE):
  fused   - single kernel; clipped x cached in SBUF as int16 (scale 4095),
            on-device AllReduce(max). One HBM read of x + one write of y.
  twopass - exact: kernel 1 reduces |x| per core, host combines the 8 maxima,
            kernel 2 re-reads x and computes y.
"""

import os
import sys

sys.path.insert(0, "/opt/trn_rl_repo")

import numpy as np
import ml_dtypes

import concourse.bass as bass
import concourse.bacc as bacc
import concourse.mybir as mybir
import concourse.tile as tile
from concourse.bass_utils import run_bass_kernel_spmd

MAGIC = 12582912.0  # 1.5 * 2^23: adding then subtracting rounds fp32 to int (RNE)
CACHE_SCALE = 4095.0  # int16 fixed-point scale for the SBUF cache of clip(x)
EPS = 1e-8
QMAX = 127.0

N_CORES = 8
NB, C, H, W = 16, 64, 256, 256
HW = H * W  # 65536
P = 128  # partitions: 2 images x 64 channels
FREE = HW  # per-core free dim
TILE_F = 2048  # columns per streamed tile
NT = FREE // TILE_F  # 16 tiles
CHUNK = 512  # matmul moving free dim

_cache = {}

fp32 = mybir.dt.float32
bf16 = mybir.dt.bfloat16
i16 = mybir.dt.int16
Alu = mybir.AluOpType
Act = mybir.ActivationFunctionType


def _weight_quant_and_scales(nc, tc, cst, psum_misc, w_d, id64_d, id2_d):
    """Builds the block-diag ternary lhsT [128,128] bf16 and returns
    (lhsT_bd, s_w column [64,1] f32)."""
    # load w twice side by side: [64, 128] f32
    w2f = cst.tile([64, 2 * C], fp32)
    nc.sync.dma_start(w2f[:, 0:C], w_d[:])
    nc.sync.dma_start(w2f[:, C : 2 * C], w_d[:])

    # s_w = max(mean|w|, eps) per output channel (partition)
    asum = cst.tile([64, 1], fp32)
    nc.vector.tensor_reduce(
        asum[:], w2f[:, 0:C], op=Alu.add, axis=mybir.AxisListType.X,
        apply_absolute_value=True,
    )
    s_w = cst.tile([64, 1], fp32)
    nc.vector.tensor_scalar(s_w[:], asum[:], 1.0 / C, EPS, Alu.mult, Alu.max)
    rw = cst.tile([64, 1], fp32)
    nc.vector.reciprocal(rw[:], s_w[:])

    # tern = clip(round(w / s_w), -1, 1) on both halves, bf16
    tq = cst.tile([64, 2 * C], fp32)
    nc.scalar.activation(tq[:], w2f[:], Act.Copy, bias=MAGIC, scale=rw[:])
    tmp = cst.tile([64, 2 * C], fp32)
    nc.vector.tensor_scalar(tmp[:], tq[:], -MAGIC, 1.0, Alu.add, Alu.min)
    tern = cst.tile([64, 2 * C], bf16)
    nc.vector.tensor_scalar(tern[:], tmp[:], -1.0, None, Alu.max)

    # transpose both copies at once: psum_t[p, j] = tern[j, p] -> [128, 64]
    id64 = cst.tile([64, 64], bf16)
    nc.sync.dma_start(id64[:], id64_d[:])
    psum_t_full = psum_misc.tile([128, 1024], fp32, tag="ps")
    psum_t = psum_t_full[:, 0:64]
    nc.tensor.matmul(psum_t[:], tern[:], id64[:])

    lhsT_bd = cst.tile([128, 128], bf16)
    nc.vector.memset(lhsT_bd[:], 0.0)
    nc.vector.tensor_copy(lhsT_bd[0:64, 0:64], psum_t[0:64, :])
    nc.vector.tensor_copy(lhsT_bd[64:128, 64:128], psum_t[64:128, :])
    return lhsT_bd, s_w


def _scale_vectors(nc, tc, cst, psum_misc, bcol, s_w, id2_d):
    """bcol [1,2] f32 holds (beta, s). Broadcast across partitions and build
    beta_vec [128,1] (quant scale) and svec [128,1] = s * s_w[p % 64]."""
    ones = cst.tile([1, 128], fp32)
    nc.vector.memset(ones[:], 1.0)
    bps_full = psum_misc.tile([128, 1024], fp32, tag="ps")
    bps = bps_full[:, 0:2]
    nc.tensor.matmul(bps[:], ones[:], bcol[:])
    bsb = cst.tile([128, 2], fp32)
    nc.scalar.copy(bsb[:], bps[:])

    id2 = cst.tile([64, 128], fp32)
    nc.sync.dma_start(id2[:], id2_d[:])
    psv_full = psum_misc.tile([128, 1024], fp32, tag="ps")
    psv = psv_full[:, 0:1]
    nc.tensor.matmul(psv[:], id2[:], s_w[:])
    svec = cst.tile([128, 1], fp32)
    nc.vector.tensor_mul(svec[:], psv[:], bsb[:, 1:2])
    return bsb, svec


def build_fused():
    """Single-launch kernel.

    pass 1 (per 2048-col tile): load x (SP/Pool DMA), reduce absmax (DVE),
      cache = int16-convert(x * 4095) on ACT — the HW convert is
      round-half-even + saturating, so this IS round(clip) in one op.
    mid: gpsimd partition-reduce, AllReduce(max), scale vectors.
    pass 2 (per tile): u8 = int8-convert(cache * beta) (DVE; RNE+saturate
      = clip(round(.), +-127)), u8 -> bf16 halves (DVE), block-diag matmul
      into 2-bank PSUM tiles (PE), evict * svec (ACT), store (Pool SWDGE).
    """
    nc = bacc.Bacc("TRN2", target_bir_lowering=False)
    x_d = nc.dram_tensor("x", [P, FREE], fp32, kind="ExternalInput")
    w_d = nc.dram_tensor("w", [64, 64], fp32, kind="ExternalInput")
    id64_d = nc.dram_tensor("id64", [64, 64], bf16, kind="ExternalInput")
    id2_d = nc.dram_tensor("id2", [64, 128], fp32, kind="ExternalInput")
    y_d = nc.dram_tensor("y", [P, FREE], fp32, kind="ExternalOutput")

    cc_in = nc.dram_tensor("cc_in", [1], fp32)
    cc_out = nc.dram_tensor("cc_out", [1], fp32, addr_space="Shared")

    i8 = mybir.dt.int8
    HALF = TILE_F // 2
    PW = 1024

    with tile.TileContext(nc) as tc:
        with (
            tc.tile_pool(name="cachep", bufs=1) as cachep,
            tc.tile_pool(name="io32", bufs=3) as io32,
            tc.tile_pool(name="u8p", bufs=2) as u8p,
            tc.tile_pool(name="ubfp", bufs=2) as ubfp,
            tc.tile_pool(name="cst", bufs=1) as cst,
            tc.tile_pool(name="psum_main", bufs=4, space="PSUM") as psum_main,
        ):
            lhsT_bd, s_w = _weight_quant_and_scales(
                nc, tc, cst, psum_main, w_d, id64_d, id2_d
            )

            cache = cachep.tile([P, FREE], i16)
            rmax_all = cst.tile([P, NT], fp32)

            # ---- pass 1 ----
            for t in range(NT):
                sl = slice(t * TILE_F, (t + 1) * TILE_F)
                xt = io32.tile([P, TILE_F], fp32, tag="io32")
                ldq = nc.sync if t % 2 == 0 else nc.gpsimd
                ldq.dma_start(xt[:], x_d[:, sl])
                nc.vector.tensor_reduce(
                    rmax_all[:, t : t + 1], xt[:], op=Alu.max,
                    axis=mybir.AxisListType.X, apply_absolute_value=True,
                )
                # int16 convert: RNE + saturate. |x|>8 saturates at 32767,
                # within 7/4095 of the exact clip value 32760.
                nc.scalar.activation(cache[:, sl], xt[:], Act.Copy, bias=0.0,
                                     scale=CACHE_SCALE)

            # ---- local max -> scalar, AllReduce(max), scales ----
            racc = cst.tile([P, 1], fp32)
            nc.vector.tensor_reduce(racc[:], rmax_all[:], op=Alu.max,
                                    axis=mybir.AxisListType.X)
            mloc = cst.tile([1, 1], fp32)
            nc.gpsimd.tensor_reduce(mloc[:], racc[:], op=Alu.max,
                                    axis=mybir.AxisListType.C)
            # m = max(min(max|x|, 8), eps)
            nc.vector.tensor_scalar(mloc[:], mloc[:], 8.0, EPS, Alu.min, Alu.max)

            nc.gpsimd.dma_start(cc_in[:], mloc[0, :])
            nc.gpsimd.collective_compute(
                "AllReduce", Alu.max,
                ins=[cc_in[:]], outs=[cc_out[:]],
                replica_groups=[list(range(N_CORES))],
            )
            mglob = cst.tile([1, 1], fp32)
            nc.gpsimd.dma_start(mglob[:], cc_out[None, :])

            rec = cst.tile([1, 1], fp32)
            nc.vector.reciprocal(rec[:], mglob[:])
            bcol = cst.tile([1, 2], fp32)
            # beta = (127 / m) / CACHE_SCALE ; s = m / 127
            nc.vector.tensor_scalar(bcol[:, 0:1], rec[:], QMAX / CACHE_SCALE,
                                    None, Alu.mult)
            nc.vector.tensor_scalar(bcol[:, 1:2], mglob[:], 1.0 / QMAX,
                                    None, Alu.mult)
            bsb, svec = _scale_vectors(nc, tc, cst, psum_main, bcol, s_w, id2_d)

            # ---- pass 2 ----
            for t in range(NT):
                sl = slice(t * TILE_F, (t + 1) * TILE_F)
                # u8 = clip(round(cache * beta), -127, 127) via RNE+sat convert
                u8 = u8p.tile([P, TILE_F], i8)
                nc.vector.tensor_scalar(u8[:], cache[:, sl], bsb[:, 0:1], None,
                                        Alu.mult)
                out_t = io32.tile([P, TILE_F], fp32, tag="io32")
                for h in range(2):
                    hsl = slice(h * HALF, (h + 1) * HALF)
                    ubf = ubfp.tile([P, HALF], bf16)
                    nc.vector.tensor_scalar(ubf[:], u8[:, hsl], 1.0, None,
                                            Alu.mult)
                    for q in range(HALF // PW):
                        ps = psum_main.tile([P, PW], fp32, tag="ps")
                        for c in range(PW // CHUNK):
                            csl = slice(q * PW + c * CHUNK, q * PW + (c + 1) * CHUNK)
                            nc.tensor.matmul(ps[:, c * CHUNK:(c + 1) * CHUNK],
                                             lhsT_bd[:], ubf[:, csl])
                        nc.scalar.activation(
                            out_t[:, h * HALF + q * PW : h * HALF + (q + 1) * PW],
                            ps[:], Act.Copy, bias=0.0, scale=svec[:])
                stq = nc.gpsimd if t % 2 == 0 else nc.sync
                stq.dma_start(y_d[:, sl], out_t[:])
    nc.compile()
    return nc


def build_maxpass():
    nc = bacc.Bacc("TRN2", target_bir_lowering=False)
    x_d = nc.dram_tensor("x", [P, FREE], fp32, kind="ExternalInput")
    m_d = nc.dram_tensor("m", [1, 1], fp32, kind="ExternalOutput")
    TF = 8192
    with tile.TileContext(nc) as tc:
        with (
            tc.tile_pool(name="io32", bufs=3) as io32,
            tc.tile_pool(name="cst", bufs=1) as cst,
        ):
            nt = FREE // TF
            rmax_all = cst.tile([P, nt], fp32)
            for t in range(nt):
                xt = io32.tile([P, TF], fp32, tag="io32")
                ldq = nc.sync if t % 2 == 0 else nc.gpsimd
                ldq.dma_start(xt[:], x_d[:, t * TF : (t + 1) * TF])
                nc.vector.tensor_reduce(
                    rmax_all[:, t : t + 1], xt[:], op=Alu.max,
                    axis=mybir.AxisListType.X, apply_absolute_value=True,
                )
            racc = cst.tile([P, 1], fp32)
            nc.vector.tensor_reduce(racc[:], rmax_all[:], op=Alu.max,
                                    axis=mybir.AxisListType.X)
            mloc = cst.tile([1, 1], fp32)
            nc.gpsimd.tensor_reduce(mloc[:], racc[:], op=Alu.max,
                                    axis=mybir.AxisListType.C)
            nc.sync.dma_start(m_d[:], mloc[:])
    nc.compile()
    return nc


def build_mainpass():
    """Exact second kernel: reads x again, quantizes with host-provided scales.

    u8 = int8-convert(x * inv_s): RNE rounding + saturation at +-127 implement
    round and both clips exactly (when max|x| > 8 the global max m is 8, and
    8 * inv_s = 127, so saturation equals the reference clip).
    """
    nc = bacc.Bacc("TRN2", target_bir_lowering=False)
    x_d = nc.dram_tensor("x", [P, FREE], fp32, kind="ExternalInput")
    w_d = nc.dram_tensor("w", [64, 64], fp32, kind="ExternalInput")
    id64_d = nc.dram_tensor("id64", [64, 64], bf16, kind="ExternalInput")
    id2_d = nc.dram_tensor("id2", [64, 128], fp32, kind="ExternalInput")
    sc_d = nc.dram_tensor("scales", [1, 2], fp32, kind="ExternalInput")
    y_d = nc.dram_tensor("y", [P, FREE], fp32, kind="ExternalOutput")
    TF = 8192
    i8 = mybir.dt.int8
    PW = 1024
    with tile.TileContext(nc) as tc:
        with (
            tc.tile_pool(name="io32", bufs=3) as io32,
            tc.tile_pool(name="u8p", bufs=2) as u8p,
            tc.tile_pool(name="ubfp", bufs=2) as ubfp,
            tc.tile_pool(name="cst", bufs=1) as cst,
            tc.tile_pool(name="psum_main", bufs=4, space="PSUM") as psum_main,
        ):
            lhsT_bd, s_w = _weight_quant_and_scales(
                nc, tc, cst, psum_main, w_d, id64_d, id2_d
            )
            bcol = cst.tile([1, 2], fp32)
            nc.sync.dma_start(bcol[:], sc_d[:])
            bsb, svec = _scale_vectors(nc, tc, cst, psum_main, bcol, s_w, id2_d)

            nt = FREE // TF
            for t in range(nt):
                sl = slice(t * TF, (t + 1) * TF)
                xt = io32.tile([P, TF], fp32, tag="io32")
                ldq = nc.sync if t % 2 == 0 else nc.gpsimd
                ldq.dma_start(xt[:], x_d[:, sl])
                u8 = u8p.tile([P, TF], i8)
                nc.vector.tensor_scalar(u8[:], xt[:], bsb[:, 0:1], None, Alu.mult)
                out_t = io32.tile([P, TF], fp32, tag="io32")
                for h in range(TF // 2048):
                    hsl = slice(h * 2048, (h + 1) * 2048)
                    ubf = ubfp.tile([P, 2048], bf16)
                    nc.vector.tensor_scalar(ubf[:], u8[:, hsl], 1.0, None,
                                            Alu.mult)
                    for q in range(2048 // PW):
                        ps = psum_main.tile([P, PW], fp32, tag="ps")
                        for c in range(PW // CHUNK):
                            csl = slice(q * PW + c * CHUNK,
                                        q * PW + (c + 1) * CHUNK)
                            nc.tensor.matmul(ps[:, c * CHUNK:(c + 1) * CHUNK],
                                             lhsT_bd[:], ubf[:, csl])
                        nc.scalar.activation(
                            out_t[:, h * 2048 + q * PW : h * 2048 + (q + 1) * PW],
                            ps[:], Act.Copy, bias=0.0, scale=svec[:])
                stq = nc.gpsimd if t % 2 == 0 else nc.sync
                stq.dma_start(y_d[:, sl], out_t[:])
    nc.compile()
    return nc


def _consts():
    id64 = np.eye(64, dtype=np.float32).astype(ml_dtypes.bfloat16)
    id2 = np.concatenate([np.eye(64, dtype=np.float32)] * 2, axis=1)  # [64,128]
    return id64, np.ascontiguousarray(id2)


_last_results = {}


def kernel(x: np.ndarray, weight: np.ndarray) -> np.ndarray:
    mode = os.environ.get("BITCONV_MODE", "fused")
    trace = os.environ.get("BITCONV_TRACE", "0") == "1"
    if not trace:
        # The NTFF profile hook is unavailable through this axon client;
        # make sure nothing engages the trace path.
        os.environ.setdefault("BASS_NEVER_TRACE", "1")
    x = np.ascontiguousarray(x, dtype=np.float32)
    w = np.ascontiguousarray(weight.reshape(64, 64), dtype=np.float32)
    id64, id2 = _consts()
    core_ids = list(range(N_CORES))
    xs = x.reshape(N_CORES, P, FREE)  # 2 images x 64 ch on partitions

    if mode == "fused":
        if "fused" not in _cache:
            _cache["fused"] = build_fused()
        nc = _cache["fused"]
        in_maps = [
            {"x": xs[i], "w": w, "id64": id64, "id2": id2} for i in core_ids
        ]
        res = run_bass_kernel_spmd(nc, in_maps, core_ids, trace=trace)
        _last_results["fused"] = res
        y = np.stack([res.results[i]["y"] for i in core_ids])
        return np.ascontiguousarray(y.reshape(NB, C, H, W), dtype=np.float32)

    # twopass (exact)
    if "maxp" not in _cache:
        _cache["maxp"] = build_maxpass()
        _cache["mainp"] = build_mainpass()
    res1 = run_bass_kernel_spmd(
        _cache["maxp"], [{"x": xs[i]} for i in core_ids], core_ids, trace=trace
    )
    _last_results["maxp"] = res1
    maxabs = float(np.max([res1.results[i]["m"] for i in core_ids]))
    m = np.float32(max(min(maxabs, 8.0), EPS))
    s = m / np.float32(QMAX)
    beta = np.float32(1.0) / s
    scales = np.array([[beta, s]], dtype=np.float32)
    in_maps = [
        {"x": xs[i], "w": w, "id64": id64, "id2": id2, "scales": scales}
        for i in core_ids
    ]
    res2 = run_bass_kernel_spmd(_cache["mainp"], in_maps, core_ids, trace=trace)
    _last_results["mainp"] = res2
    y = np.stack([res2.results[i]["y"] for i in core_ids])
    return np.ascontiguousarray(y.reshape(NB, C, H, W), dtype=np.float32)


# revision 27
# speedup vs baseline: 1.0367x; 1.0367x over previous
"""Trainium2 Bass kernel for BitConv2d (act fake-quant int8 + ternary weight 1x1 conv).

Full inputs: x (16, 64, 256, 256) f32, weight (64, 64, 1, 1) f32.
Sharded data-parallel over 8 NeuronCores: 2 batch images per core.
Per-core view of x is [128, 65536]: partition p = n*64 + c, free = h*w.

Math (matches reference):
  xc    = clip(x, -8, 8)
  m     = max(|xc|) over the WHOLE tensor      (global -> AllReduce max)
  s     = max(m, 1e-8) / 127
  u     = round(xc / s)                         (integers in [-127, 127])
  wq    = clip(round(w / s_w), -1, 1) * s_w,    s_w = max(mean|w|, 1e-8) per out-ch
  y     = conv1x1(u * s, wq) = (s * s_w[o]) * (tern @ u)

The integer matmul tern @ u is exact in bf16 x bf16 with fp32 PSUM accumulate.
Rounding uses the hardware float->int conversions, which are
round-half-to-even (= jnp.round) with saturation (= the reference clips).

Modes (env BITCONV_MODE):
  fused   - single kernel; clipped x cached in SBUF as int16 (scale 4095),
            on-device AllReduce(max). One HBM read of x + one write of y.
  twopass - exact: kernel 1 reduces |x| per core, host combines the 8 maxima,
            kernel 2 re-reads x and computes y.
"""

import os
import sys

sys.path.insert(0, "/opt/trn_rl_repo")

import numpy as np
import ml_dtypes

import concourse.bass as bass
import concourse.bacc as bacc
import concourse.mybir as mybir
import concourse.tile as tile
from concourse.bass_utils import run_bass_kernel_spmd

MAGIC = 12582912.0  # 1.5 * 2^23: adding then subtracting rounds fp32 to int (RNE)
CACHE_SCALE = 4095.0  # int16 fixed-point scale for the SBUF cache of clip(x)
EPS = 1e-8
QMAX = 127.0

N_CORES = 8
NB, C, H, W = 16, 64, 256, 256
HW = H * W  # 65536
P = 128  # partitions: 2 images x 64 channels
FREE = HW  # per-core free dim
TILE_F = 2048  # columns per streamed tile
NT = FREE // TILE_F  # 16 tiles
CHUNK = 512  # matmul moving free dim

_cache = {}

fp32 = mybir.dt.float32
bf16 = mybir.dt.bfloat16
i16 = mybir.dt.int16
Alu = mybir.AluOpType
Act = mybir.ActivationFunctionType


def _weight_quant_and_scales(nc, tc, cst, psum_misc, w_d, id64_d, id2_d):
    """Builds the block-diag ternary lhsT [128,128] bf16 and returns
    (lhsT_bd, s_w column [64,1] f32)."""
    # load w twice side by side: [64, 128] f32
    w2f = cst.tile([64, 2 * C], fp32)
    nc.sync.dma_start(w2f[:, 0:C], w_d[:])
    nc.sync.dma_start(w2f[:, C : 2 * C], w_d[:])

    # s_w = max(mean|w|, eps) per output channel (partition)
    asum = cst.tile([64, 1], fp32)
    nc.vector.tensor_reduce(
        asum[:], w2f[:, 0:C], op=Alu.add, axis=mybir.AxisListType.X,
        apply_absolute_value=True,
    )
    s_w = cst.tile([64, 1], fp32)
    nc.vector.tensor_scalar(s_w[:], asum[:], 1.0 / C, EPS, Alu.mult, Alu.max)
    rw = cst.tile([64, 1], fp32)
    nc.vector.reciprocal(rw[:], s_w[:])

    # tern = clip(round(w / s_w), -1, 1) on both halves, bf16
    tq = cst.tile([64, 2 * C], fp32)
    nc.scalar.activation(tq[:], w2f[:], Act.Copy, bias=MAGIC, scale=rw[:])
    tmp = cst.tile([64, 2 * C], fp32)
    nc.vector.tensor_scalar(tmp[:], tq[:], -MAGIC, 1.0, Alu.add, Alu.min)
    tern = cst.tile([64, 2 * C], bf16)
    nc.vector.tensor_scalar(tern[:], tmp[:], -1.0, None, Alu.max)

    # transpose both copies at once: psum_t[p, j] = tern[j, p] -> [128, 64]
    id64 = cst.tile([64, 64], bf16)
    nc.sync.dma_start(id64[:], id64_d[:])
    psum_t_full = psum_misc.tile([128, 1024], fp32, tag="ps")
    psum_t = psum_t_full[:, 0:64]
    nc.tensor.matmul(psum_t[:], tern[:], id64[:])

    lhsT_bd = cst.tile([128, 128], bf16)
    nc.vector.memset(lhsT_bd[:], 0.0)
    nc.vector.tensor_copy(lhsT_bd[0:64, 0:64], psum_t[0:64, :])
    nc.vector.tensor_copy(lhsT_bd[64:128, 64:128], psum_t[64:128, :])
    return lhsT_bd, s_w


def _scale_vectors(nc, tc, cst, psum_misc, bcol, s_w, id2_d):
    """bcol [1,2] f32 holds (beta, s). Broadcast across partitions and build
    beta_vec [128,1] (quant scale) and svec [128,1] = s * s_w[p % 64]."""
    ones = cst.tile([1, 128], fp32)
    nc.vector.memset(ones[:], 1.0)
    bps_full = psum_misc.tile([128, 1024], fp32, tag="ps")
    bps = bps_full[:, 0:2]
    nc.tensor.matmul(bps[:], ones[:], bcol[:])
    bsb = cst.tile([128, 2], fp32)
    nc.scalar.copy(bsb[:], bps[:])

    id2 = cst.tile([64, 128], fp32)
    nc.sync.dma_start(id2[:], id2_d[:])
    psv_full = psum_misc.tile([128, 1024], fp32, tag="ps")
    psv = psv_full[:, 0:1]
    nc.tensor.matmul(psv[:], id2[:], s_w[:])
    svec = cst.tile([128, 1], fp32)
    nc.vector.tensor_mul(svec[:], psv[:], bsb[:, 1:2])
    return bsb, svec


def build_fused():
    """Single-launch kernel.

    pass 1 (per 2048-col tile): load x (SP/Pool DMA), reduce absmax (DVE),
      cache = int16-convert(x * 4095) on ACT — the HW convert is
      round-half-even + saturating, so this IS round(clip) in one op.
    mid: gpsimd partition-reduce, AllReduce(max), scale vectors.
    pass 2 (per tile): u8 = int8-convert(cache * beta) (DVE; RNE+saturate
      = clip(round(.), +-127)), u8 -> bf16 halves (DVE), block-diag matmul
      into 2-bank PSUM tiles (PE), evict * svec (ACT), store (Pool SWDGE).
    """
    nc = bacc.Bacc("TRN2", target_bir_lowering=False)
    x_d = nc.dram_tensor("x", [P, FREE], fp32, kind="ExternalInput")
    w_d = nc.dram_tensor("w", [64, 64], fp32, kind="ExternalInput")
    id64_d = nc.dram_tensor("id64", [64, 64], bf16, kind="ExternalInput")
    id2_d = nc.dram_tensor("id2", [64, 128], fp32, kind="ExternalInput")
    y_d = nc.dram_tensor("y", [P, FREE], fp32, kind="ExternalOutput")

    cc_in = nc.dram_tensor("cc_in", [1], fp32)
    cc_out = nc.dram_tensor("cc_out", [1], fp32, addr_space="Shared")

    i8 = mybir.dt.int8
    HALF = TILE_F // 2
    PW = 1024

    with tile.TileContext(nc) as tc:
        with (
            tc.tile_pool(name="cachep", bufs=1) as cachep,
            tc.tile_pool(name="io32", bufs=3) as io32,
            tc.tile_pool(name="u8p", bufs=2) as u8p,
            tc.tile_pool(name="ubfp", bufs=2) as ubfp,
            tc.tile_pool(name="cst", bufs=1) as cst,
            tc.tile_pool(name="psum_main", bufs=4, space="PSUM") as psum_main,
        ):
            lhsT_bd, s_w = _weight_quant_and_scales(
                nc, tc, cst, psum_main, w_d, id64_d, id2_d
            )

            cache = cachep.tile([P, FREE], i16)
            rmax_all = cst.tile([P, NT], fp32)

            # ---- pass 1 ----
            for t in range(NT):
                sl = slice(t * TILE_F, (t + 1) * TILE_F)
                xt = io32.tile([P, TILE_F], fp32, tag="io32")
                ldq = nc.sync if t % 2 == 0 else nc.gpsimd
                ldq.dma_start(xt[:], x_d[:, sl])
                nc.vector.tensor_reduce(
                    rmax_all[:, t : t + 1], xt[:], op=Alu.max,
                    axis=mybir.AxisListType.X, apply_absolute_value=True,
                )
                # int16 convert: RNE + saturate. |x|>8 saturates at 32767,
                # within 7/4095 of the exact clip value 32760.
                nc.scalar.activation(cache[:, sl], xt[:], Act.Copy, bias=0.0,
                                     scale=CACHE_SCALE)

            # ---- local max -> scalar, AllReduce(max), scales ----
            racc = cst.tile([P, 1], fp32)
            nc.vector.tensor_reduce(racc[:], rmax_all[:], op=Alu.max,
                                    axis=mybir.AxisListType.X)
            mloc = cst.tile([1, 1], fp32)
            nc.gpsimd.tensor_reduce(mloc[:], racc[:], op=Alu.max,
                                    axis=mybir.AxisListType.C)
            # m = max(min(max|x|, 8), eps)
            nc.vector.tensor_scalar(mloc[:], mloc[:], 8.0, EPS, Alu.min, Alu.max)

            nc.gpsimd.dma_start(cc_in[:], mloc[0, :])
            nc.gpsimd.collective_compute(
                "AllReduce", Alu.max,
                ins=[cc_in[:]], outs=[cc_out[:]],
                replica_groups=[list(range(N_CORES))],
            )
            mglob = cst.tile([1, 1], fp32)
            nc.gpsimd.dma_start(mglob[:], cc_out[None, :])

            rec = cst.tile([1, 1], fp32)
            nc.vector.reciprocal(rec[:], mglob[:])
            bcol = cst.tile([1, 2], fp32)
            # beta = (127 / m) / CACHE_SCALE ; s = m / 127
            nc.vector.tensor_scalar(bcol[:, 0:1], rec[:], QMAX / CACHE_SCALE,
                                    None, Alu.mult)
            nc.vector.tensor_scalar(bcol[:, 1:2], mglob[:], 1.0 / QMAX,
                                    None, Alu.mult)
            bsb, svec = _scale_vectors(nc, tc, cst, psum_main, bcol, s_w, id2_d)

            # ---- pass 2 ----
            for t in range(NT):
                sl = slice(t * TILE_F, (t + 1) * TILE_F)
                # u8 = clip(round(cache * beta), -127, 127) via RNE+sat convert
                u8 = u8p.tile([P, TILE_F], i8)
                nc.vector.tensor_scalar(u8[:], cache[:, sl], bsb[:, 0:1], None,
                                        Alu.mult)
                out_t = io32.tile([P, TILE_F], fp32, tag="io32")
                for h in range(2):
                    hsl = slice(h * HALF, (h + 1) * HALF)
                    ubf = ubfp.tile([P, HALF], bf16)
                    ueng = nc.gpsimd if (t % 2 == 1 and h == 1) else nc.vector
                    ueng.tensor_scalar(ubf[:], u8[:, hsl], 1.0, None,
                                       Alu.mult)
                    for q in range(HALF // PW):
                        ps = psum_main.tile([P, PW], fp32, tag="ps")
                        for c in range(PW // CHUNK):
                            csl = slice(q * PW + c * CHUNK, q * PW + (c + 1) * CHUNK)
                            nc.tensor.matmul(ps[:, c * CHUNK:(c + 1) * CHUNK],
                                             lhsT_bd[:], ubf[:, csl])
                        nc.scalar.activation(
                            out_t[:, h * HALF + q * PW : h * HALF + (q + 1) * PW],
                            ps[:], Act.Copy, bias=0.0, scale=svec[:])
                stq = nc.gpsimd if t % 2 == 0 else nc.sync
                stq.dma_start(y_d[:, sl], out_t[:])
    nc.compile()
    return nc


def build_maxpass():
    nc = bacc.Bacc("TRN2", target_bir_lowering=False)
    x_d = nc.dram_tensor("x", [P, FREE], fp32, kind="ExternalInput")
    m_d = nc.dram_tensor("m", [1, 1], fp32, kind="ExternalOutput")
    TF = 8192
    with tile.TileContext(nc) as tc:
        with (
            tc.tile_pool(name="io32", bufs=3) as io32,
            tc.tile_pool(name="cst", bufs=1) as cst,
        ):
            nt = FREE // TF
            rmax_all = cst.tile([P, nt], fp32)
            for t in range(nt):
                xt = io32.tile([P, TF], fp32, tag="io32")
                ldq = nc.sync if t % 2 == 0 else nc.gpsimd
                ldq.dma_start(xt[:], x_d[:, t * TF : (t + 1) * TF])
                nc.vector.tensor_reduce(
                    rmax_all[:, t : t + 1], xt[:], op=Alu.max,
                    axis=mybir.AxisListType.X, apply_absolute_value=True,
                )
            racc = cst.tile([P, 1], fp32)
            nc.vector.tensor_reduce(racc[:], rmax_all[:], op=Alu.max,
                                    axis=mybir.AxisListType.X)
            mloc = cst.tile([1, 1], fp32)
            nc.gpsimd.tensor_reduce(mloc[:], racc[:], op=Alu.max,
                                    axis=mybir.AxisListType.C)
            nc.sync.dma_start(m_d[:], mloc[:])
    nc.compile()
    return nc


def build_mainpass():
    """Exact second kernel: reads x again, quantizes with host-provided scales.

    u8 = int8-convert(x * inv_s): RNE rounding + saturation at +-127 implement
    round and both clips exactly (when max|x| > 8 the global max m is 8, and
    8 * inv_s = 127, so saturation equals the reference clip).
    """
    nc = bacc.Bacc("TRN2", target_bir_lowering=False)
    x_d = nc.dram_tensor("x", [P, FREE], fp32, kind="ExternalInput")
    w_d = nc.dram_tensor("w", [64, 64], fp32, kind="ExternalInput")
    id64_d = nc.dram_tensor("id64", [64, 64], bf16, kind="ExternalInput")
    id2_d = nc.dram_tensor("id2", [64, 128], fp32, kind="ExternalInput")
    sc_d = nc.dram_tensor("scales", [1, 2], fp32, kind="ExternalInput")
    y_d = nc.dram_tensor("y", [P, FREE], fp32, kind="ExternalOutput")
    TF = 8192
    i8 = mybir.dt.int8
    PW = 1024
    with tile.TileContext(nc) as tc:
        with (
            tc.tile_pool(name="io32", bufs=3) as io32,
            tc.tile_pool(name="u8p", bufs=2) as u8p,
            tc.tile_pool(name="ubfp", bufs=2) as ubfp,
            tc.tile_pool(name="cst", bufs=1) as cst,
            tc.tile_pool(name="psum_main", bufs=4, space="PSUM") as psum_main,
        ):
            lhsT_bd, s_w = _weight_quant_and_scales(
                nc, tc, cst, psum_main, w_d, id64_d, id2_d
            )
            bcol = cst.tile([1, 2], fp32)
            nc.sync.dma_start(bcol[:], sc_d[:])
            bsb, svec = _scale_vectors(nc, tc, cst, psum_main, bcol, s_w, id2_d)

            nt = FREE // TF
            for t in range(nt):
                sl = slice(t * TF, (t + 1) * TF)
                xt = io32.tile([P, TF], fp32, tag="io32")
                ldq = nc.sync if t % 2 == 0 else nc.gpsimd
                ldq.dma_start(xt[:], x_d[:, sl])
                u8 = u8p.tile([P, TF], i8)
                nc.vector.tensor_scalar(u8[:], xt[:], bsb[:, 0:1], None, Alu.mult)
                out_t = io32.tile([P, TF], fp32, tag="io32")
                for h in range(TF // 2048):
                    hsl = slice(h * 2048, (h + 1) * 2048)
                    ubf = ubfp.tile([P, 2048], bf16)
                    nc.vector.tensor_scalar(ubf[:], u8[:, hsl], 1.0, None,
                                            Alu.mult)
                    for q in range(2048 // PW):
                        ps = psum_main.tile([P, PW], fp32, tag="ps")
                        for c in range(PW // CHUNK):
                            csl = slice(q * PW + c * CHUNK,
                                        q * PW + (c + 1) * CHUNK)
                            nc.tensor.matmul(ps[:, c * CHUNK:(c + 1) * CHUNK],
                                             lhsT_bd[:], ubf[:, csl])
                        nc.scalar.activation(
                            out_t[:, h * 2048 + q * PW : h * 2048 + (q + 1) * PW],
                            ps[:], Act.Copy, bias=0.0, scale=svec[:])
                stq = nc.gpsimd if t % 2 == 0 else nc.sync
                stq.dma_start(y_d[:, sl], out_t[:])
    nc.compile()
    return nc


def _consts():
    id64 = np.eye(64, dtype=np.float32).astype(ml_dtypes.bfloat16)
    id2 = np.concatenate([np.eye(64, dtype=np.float32)] * 2, axis=1)  # [64,128]
    return id64, np.ascontiguousarray(id2)


_last_results = {}


def kernel(x: np.ndarray, weight: np.ndarray) -> np.ndarray:
    mode = os.environ.get("BITCONV_MODE", "fused")
    trace = os.environ.get("BITCONV_TRACE", "0") == "1"
    if not trace:
        # The NTFF profile hook is unavailable through this axon client;
        # make sure nothing engages the trace path.
        os.environ.setdefault("BASS_NEVER_TRACE", "1")
    x = np.ascontiguousarray(x, dtype=np.float32)
    w = np.ascontiguousarray(weight.reshape(64, 64), dtype=np.float32)
    id64, id2 = _consts()
    core_ids = list(range(N_CORES))
    xs = x.reshape(N_CORES, P, FREE)  # 2 images x 64 ch on partitions

    if mode == "fused":
        if "fused" not in _cache:
            _cache["fused"] = build_fused()
        nc = _cache["fused"]
        in_maps = [
            {"x": xs[i], "w": w, "id64": id64, "id2": id2} for i in core_ids
        ]
        res = run_bass_kernel_spmd(nc, in_maps, core_ids, trace=trace)
        _last_results["fused"] = res
        y = np.stack([res.results[i]["y"] for i in core_ids])
        return np.ascontiguousarray(y.reshape(NB, C, H, W), dtype=np.float32)

    # twopass (exact)
    if "maxp" not in _cache:
        _cache["maxp"] = build_maxpass()
        _cache["mainp"] = build_mainpass()
    res1 = run_bass_kernel_spmd(
        _cache["maxp"], [{"x": xs[i]} for i in core_ids], core_ids, trace=trace
    )
    _last_results["maxp"] = res1
    maxabs = float(np.max([res1.results[i]["m"] for i in core_ids]))
    m = np.float32(max(min(maxabs, 8.0), EPS))
    s = m / np.float32(QMAX)
    beta = np.float32(1.0) / s
    scales = np.array([[beta, s]], dtype=np.float32)
    in_maps = [
        {"x": xs[i], "w": w, "id64": id64, "id2": id2, "scales": scales}
        for i in core_ids
    ]
    res2 = run_bass_kernel_spmd(_cache["mainp"], in_maps, core_ids, trace=trace)
    _last_results["mainp"] = res2
    y = np.stack([res2.results[i]["y"] for i in core_ids])
    return np.ascontiguousarray(y.reshape(NB, C, H, W), dtype=np.float32)


# revision 28
# speedup vs baseline: 1.1164x; 1.0768x over previous
"""Trainium2 Bass kernel for BitConv2d (act fake-quant int8 + ternary weight 1x1 conv).

Full inputs: x (16, 64, 256, 256) f32, weight (64, 64, 1, 1) f32.
Sharded data-parallel over 8 NeuronCores: 2 batch images per core.
Per-core view of x is [128, 65536]: partition p = n*64 + c, free = h*w.

Math (matches reference):
  xc    = clip(x, -8, 8)
  m     = max(|xc|) over the WHOLE tensor      (global -> AllReduce max)
  s     = max(m, 1e-8) / 127
  u     = round(xc / s)                         (integers in [-127, 127])
  wq    = clip(round(w / s_w), -1, 1) * s_w,    s_w = max(mean|w|, 1e-8) per out-ch
  y     = conv1x1(u * s, wq) = (s * s_w[o]) * (tern @ u)

The integer matmul tern @ u is exact in bf16 x bf16 with fp32 PSUM accumulate.
Rounding uses the hardware float->int conversions, which are
round-half-to-even (= jnp.round) with saturation (= the reference clips).

Modes (env BITCONV_MODE):
  fused   - single kernel; clipped x cached in SBUF as int16 (scale 4095),
            on-device AllReduce(max). One HBM read of x + one write of y.
  twopass - exact: kernel 1 reduces |x| per core, host combines the 8 maxima,
            kernel 2 re-reads x and computes y.
"""

import os
import sys

sys.path.insert(0, "/opt/trn_rl_repo")

import numpy as np
import ml_dtypes

import concourse.bass as bass
import concourse.bacc as bacc
import concourse.mybir as mybir
import concourse.tile as tile
from concourse.bass_utils import run_bass_kernel_spmd

MAGIC = 12582912.0  # 1.5 * 2^23: adding then subtracting rounds fp32 to int (RNE)
CACHE_SCALE = 4095.0  # int16 fixed-point scale for the SBUF cache of clip(x)
EPS = 1e-8
QMAX = 127.0

N_CORES = 8
NB, C, H, W = 16, 64, 256, 256
HW = H * W  # 65536
P = 128  # partitions: 2 images x 64 channels
FREE = HW  # per-core free dim
TILE_F = 2048  # columns per streamed tile
NT = FREE // TILE_F  # 16 tiles
CHUNK = 512  # matmul moving free dim

_cache = {}

fp32 = mybir.dt.float32
bf16 = mybir.dt.bfloat16
i16 = mybir.dt.int16
Alu = mybir.AluOpType
Act = mybir.ActivationFunctionType


def _weight_quant_and_scales(nc, tc, cst, psum_misc, w_d, id64_d, id2_d):
    """Builds the block-diag ternary lhsT [128,128] bf16 and returns
    (lhsT_bd, s_w column [64,1] f32)."""
    # load w twice side by side: [64, 128] f32
    w2f = cst.tile([64, 2 * C], fp32)
    nc.sync.dma_start(w2f[:, 0:C], w_d[:])
    nc.sync.dma_start(w2f[:, C : 2 * C], w_d[:])

    # s_w = max(mean|w|, eps) per output channel (partition)
    asum = cst.tile([64, 1], fp32)
    nc.vector.tensor_reduce(
        asum[:], w2f[:, 0:C], op=Alu.add, axis=mybir.AxisListType.X,
        apply_absolute_value=True,
    )
    s_w = cst.tile([64, 1], fp32)
    nc.vector.tensor_scalar(s_w[:], asum[:], 1.0 / C, EPS, Alu.mult, Alu.max)
    rw = cst.tile([64, 1], fp32)
    nc.vector.reciprocal(rw[:], s_w[:])

    # tern = clip(round(w / s_w), -1, 1) on both halves, bf16
    tq = cst.tile([64, 2 * C], fp32)
    nc.scalar.activation(tq[:], w2f[:], Act.Copy, bias=MAGIC, scale=rw[:])
    tmp = cst.tile([64, 2 * C], fp32)
    nc.vector.tensor_scalar(tmp[:], tq[:], -MAGIC, 1.0, Alu.add, Alu.min)
    tern = cst.tile([64, 2 * C], bf16)
    nc.vector.tensor_scalar(tern[:], tmp[:], -1.0, None, Alu.max)

    # transpose both copies at once: psum_t[p, j] = tern[j, p] -> [128, 64]
    id64 = cst.tile([64, 64], bf16)
    nc.sync.dma_start(id64[:], id64_d[:])
    psum_t_full = psum_misc.tile([128, 1024], fp32, tag="ps")
    psum_t = psum_t_full[:, 0:64]
    nc.tensor.matmul(psum_t[:], tern[:], id64[:])

    lhsT_bd = cst.tile([128, 128], bf16)
    nc.vector.memset(lhsT_bd[:], 0.0)
    nc.vector.tensor_copy(lhsT_bd[0:64, 0:64], psum_t[0:64, :])
    nc.vector.tensor_copy(lhsT_bd[64:128, 64:128], psum_t[64:128, :])
    return lhsT_bd, s_w


def _scale_vectors(nc, tc, cst, psum_misc, bcol, s_w, id2_d):
    """bcol [1,2] f32 holds (beta, s). Broadcast across partitions and build
    beta_vec [128,1] (quant scale) and svec [128,1] = s * s_w[p % 64]."""
    ones = cst.tile([1, 128], fp32)
    nc.vector.memset(ones[:], 1.0)
    bps_full = psum_misc.tile([128, 1024], fp32, tag="ps")
    bps = bps_full[:, 0:2]
    nc.tensor.matmul(bps[:], ones[:], bcol[:])
    bsb = cst.tile([128, 2], fp32)
    nc.scalar.copy(bsb[:], bps[:])

    id2 = cst.tile([64, 128], fp32)
    nc.sync.dma_start(id2[:], id2_d[:])
    psv_full = psum_misc.tile([128, 1024], fp32, tag="ps")
    psv = psv_full[:, 0:1]
    nc.tensor.matmul(psv[:], id2[:], s_w[:])
    svec = cst.tile([128, 1], fp32)
    nc.vector.tensor_mul(svec[:], psv[:], bsb[:, 1:2])
    return bsb, svec


def build_fused():
    """Single-launch kernel.

    pass 1 (per 2048-col tile): load x (SP/Pool DMA), reduce absmax (DVE),
      cache = int16-convert(x * 4095) on ACT — the HW convert is
      round-half-even + saturating, so this IS round(clip) in one op.
    mid: gpsimd partition-reduce, AllReduce(max), scale vectors.
    pass 2 (per tile): u8 = int8-convert(cache * beta) (DVE; RNE+saturate
      = clip(round(.), +-127)), u8 -> bf16 halves (DVE), block-diag matmul
      into 2-bank PSUM tiles (PE), evict * svec (ACT), store (Pool SWDGE).
    """
    nc = bacc.Bacc("TRN2", target_bir_lowering=False)
    x_d = nc.dram_tensor("x", [P, FREE], fp32, kind="ExternalInput")
    w_d = nc.dram_tensor("w", [64, 64], fp32, kind="ExternalInput")
    id64_d = nc.dram_tensor("id64", [64, 64], bf16, kind="ExternalInput")
    id2_d = nc.dram_tensor("id2", [64, 128], fp32, kind="ExternalInput")
    y_d = nc.dram_tensor("y", [P, FREE], fp32, kind="ExternalOutput")

    cc_in = nc.dram_tensor("cc_in", [1], fp32)
    cc_out = nc.dram_tensor("cc_out", [N_CORES], fp32, addr_space="Shared")

    i8 = mybir.dt.int8
    HALF = TILE_F // 2
    PW = 1024

    with tile.TileContext(nc) as tc:
        with (
            tc.tile_pool(name="cachep", bufs=1) as cachep,
            tc.tile_pool(name="io32", bufs=3) as io32,
            tc.tile_pool(name="u8p", bufs=2) as u8p,
            tc.tile_pool(name="ubfp", bufs=2) as ubfp,
            tc.tile_pool(name="cst", bufs=1) as cst,
            tc.tile_pool(name="psum_main", bufs=4, space="PSUM") as psum_main,
        ):
            lhsT_bd, s_w = _weight_quant_and_scales(
                nc, tc, cst, psum_main, w_d, id64_d, id2_d
            )

            cache = cachep.tile([P, FREE], i16)
            rmax_all = cst.tile([P, NT], fp32)

            # ---- pass 1 ----
            for t in range(NT):
                sl = slice(t * TILE_F, (t + 1) * TILE_F)
                xt = io32.tile([P, TILE_F], fp32, tag="io32")
                ldq = nc.sync if t % 2 == 0 else nc.gpsimd
                ldq.dma_start(xt[:], x_d[:, sl])
                nc.vector.tensor_reduce(
                    rmax_all[:, t : t + 1], xt[:], op=Alu.max,
                    axis=mybir.AxisListType.X, apply_absolute_value=True,
                )
                # int16 convert: RNE + saturate. |x|>8 saturates at 32767,
                # within 7/4095 of the exact clip value 32760.
                nc.scalar.activation(cache[:, sl], xt[:], Act.Copy, bias=0.0,
                                     scale=CACHE_SCALE)

            # ---- local max -> scalar, AllReduce(max), scales ----
            racc = cst.tile([P, 1], fp32)
            nc.vector.tensor_reduce(racc[:], rmax_all[:], op=Alu.max,
                                    axis=mybir.AxisListType.X)
            mloc = cst.tile([1, 1], fp32)
            nc.gpsimd.tensor_reduce(mloc[:], racc[:], op=Alu.max,
                                    axis=mybir.AxisListType.C)
            # m = max(min(max|x|, 8), eps)
            nc.vector.tensor_scalar(mloc[:], mloc[:], 8.0, EPS, Alu.min, Alu.max)

            nc.gpsimd.dma_start(cc_in[:], mloc[0, :])
            # AllGather of the 8 per-core maxima + local max: cheaper than
            # AllReduce (pure copy, no CCE hop)
            nc.gpsimd.collective_compute(
                "AllGather", Alu.bypass,
                ins=[cc_in[:]], outs=[cc_out[:]],
                replica_groups=[list(range(N_CORES))],
            )
            mrow = cst.tile([1, N_CORES], fp32)
            nc.gpsimd.dma_start(mrow[:], cc_out[None, :])
            mglob = cst.tile([1, 1], fp32)
            nc.vector.tensor_reduce(mglob[:], mrow[:], op=Alu.max,
                                    axis=mybir.AxisListType.X)

            rec = cst.tile([1, 1], fp32)
            nc.vector.reciprocal(rec[:], mglob[:])
            bcol = cst.tile([1, 2], fp32)
            # beta = (127 / m) / CACHE_SCALE ; s = m / 127
            nc.vector.tensor_scalar(bcol[:, 0:1], rec[:], QMAX / CACHE_SCALE,
                                    None, Alu.mult)
            nc.vector.tensor_scalar(bcol[:, 1:2], mglob[:], 1.0 / QMAX,
                                    None, Alu.mult)
            bsb, svec = _scale_vectors(nc, tc, cst, psum_main, bcol, s_w, id2_d)

            # ---- pass 2 ----
            for t in range(NT):
                sl = slice(t * TILE_F, (t + 1) * TILE_F)
                # u8 = clip(round(cache * beta), -127, 127) via RNE+sat convert
                u8 = u8p.tile([P, TILE_F], i8)
                nc.vector.tensor_scalar(u8[:], cache[:, sl], bsb[:, 0:1], None,
                                        Alu.mult)
                out_t = io32.tile([P, TILE_F], fp32, tag="io32")
                for h in range(2):
                    hsl = slice(h * HALF, (h + 1) * HALF)
                    ubf = ubfp.tile([P, HALF], bf16)
                    ueng = nc.gpsimd if (t % 2 == 1 and h == 1) else nc.vector
                    ueng.tensor_scalar(ubf[:], u8[:, hsl], 1.0, None,
                                       Alu.mult)
                    for q in range(HALF // PW):
                        ps = psum_main.tile([P, PW], fp32, tag="ps")
                        for c in range(PW // CHUNK):
                            csl = slice(q * PW + c * CHUNK, q * PW + (c + 1) * CHUNK)
                            nc.tensor.matmul(ps[:, c * CHUNK:(c + 1) * CHUNK],
                                             lhsT_bd[:], ubf[:, csl])
                        nc.scalar.activation(
                            out_t[:, h * HALF + q * PW : h * HALF + (q + 1) * PW],
                            ps[:], Act.Copy, bias=0.0, scale=svec[:])
                stq = nc.gpsimd if t % 2 == 0 else nc.sync
                stq.dma_start(y_d[:, sl], out_t[:])
    nc.compile()
    return nc


def build_maxpass():
    nc = bacc.Bacc("TRN2", target_bir_lowering=False)
    x_d = nc.dram_tensor("x", [P, FREE], fp32, kind="ExternalInput")
    m_d = nc.dram_tensor("m", [1, 1], fp32, kind="ExternalOutput")
    TF = 8192
    with tile.TileContext(nc) as tc:
        with (
            tc.tile_pool(name="io32", bufs=3) as io32,
            tc.tile_pool(name="cst", bufs=1) as cst,
        ):
            nt = FREE // TF
            rmax_all = cst.tile([P, nt], fp32)
            for t in range(nt):
                xt = io32.tile([P, TF], fp32, tag="io32")
                ldq = nc.sync if t % 2 == 0 else nc.gpsimd
                ldq.dma_start(xt[:], x_d[:, t * TF : (t + 1) * TF])
                nc.vector.tensor_reduce(
                    rmax_all[:, t : t + 1], xt[:], op=Alu.max,
                    axis=mybir.AxisListType.X, apply_absolute_value=True,
                )
            racc = cst.tile([P, 1], fp32)
            nc.vector.tensor_reduce(racc[:], rmax_all[:], op=Alu.max,
                                    axis=mybir.AxisListType.X)
            mloc = cst.tile([1, 1], fp32)
            nc.gpsimd.tensor_reduce(mloc[:], racc[:], op=Alu.max,
                                    axis=mybir.AxisListType.C)
            nc.sync.dma_start(m_d[:], mloc[:])
    nc.compile()
    return nc


def build_mainpass():
    """Exact second kernel: reads x again, quantizes with host-provided scales.

    u8 = int8-convert(x * inv_s): RNE rounding + saturation at +-127 implement
    round and both clips exactly (when max|x| > 8 the global max m is 8, and
    8 * inv_s = 127, so saturation equals the reference clip).
    """
    nc = bacc.Bacc("TRN2", target_bir_lowering=False)
    x_d = nc.dram_tensor("x", [P, FREE], fp32, kind="ExternalInput")
    w_d = nc.dram_tensor("w", [64, 64], fp32, kind="ExternalInput")
    id64_d = nc.dram_tensor("id64", [64, 64], bf16, kind="ExternalInput")
    id2_d = nc.dram_tensor("id2", [64, 128], fp32, kind="ExternalInput")
    sc_d = nc.dram_tensor("scales", [1, 2], fp32, kind="ExternalInput")
    y_d = nc.dram_tensor("y", [P, FREE], fp32, kind="ExternalOutput")
    TF = 8192
    i8 = mybir.dt.int8
    PW = 1024
    with tile.TileContext(nc) as tc:
        with (
            tc.tile_pool(name="io32", bufs=3) as io32,
            tc.tile_pool(name="u8p", bufs=2) as u8p,
            tc.tile_pool(name="ubfp", bufs=2) as ubfp,
            tc.tile_pool(name="cst", bufs=1) as cst,
            tc.tile_pool(name="psum_main", bufs=4, space="PSUM") as psum_main,
        ):
            lhsT_bd, s_w = _weight_quant_and_scales(
                nc, tc, cst, psum_main, w_d, id64_d, id2_d
            )
            bcol = cst.tile([1, 2], fp32)
            nc.sync.dma_start(bcol[:], sc_d[:])
            bsb, svec = _scale_vectors(nc, tc, cst, psum_main, bcol, s_w, id2_d)

            nt = FREE // TF
            for t in range(nt):
                sl = slice(t * TF, (t + 1) * TF)
                xt = io32.tile([P, TF], fp32, tag="io32")
                ldq = nc.sync if t % 2 == 0 else nc.gpsimd
                ldq.dma_start(xt[:], x_d[:, sl])
                u8 = u8p.tile([P, TF], i8)
                nc.vector.tensor_scalar(u8[:], xt[:], bsb[:, 0:1], None, Alu.mult)
                out_t = io32.tile([P, TF], fp32, tag="io32")
                for h in range(TF // 2048):
                    hsl = slice(h * 2048, (h + 1) * 2048)
                    ubf = ubfp.tile([P, 2048], bf16)
                    nc.vector.tensor_scalar(ubf[:], u8[:, hsl], 1.0, None,
                                            Alu.mult)
                    for q in range(2048 // PW):
                        ps = psum_main.tile([P, PW], fp32, tag="ps")
                        for c in range(PW // CHUNK):
                            csl = slice(q * PW + c * CHUNK,
                                        q * PW + (c + 1) * CHUNK)
                            nc.tensor.matmul(ps[:, c * CHUNK:(c + 1) * CHUNK],
                                             lhsT_bd[:], ubf[:, csl])
                        nc.scalar.activation(
                            out_t[:, h * 2048 + q * PW : h * 2048 + (q + 1) * PW],
                            ps[:], Act.Copy, bias=0.0, scale=svec[:])
                stq = nc.gpsimd if t % 2 == 0 else nc.sync
                stq.dma_start(y_d[:, sl], out_t[:])
    nc.compile()
    return nc


def _consts():
    id64 = np.eye(64, dtype=np.float32).astype(ml_dtypes.bfloat16)
    id2 = np.concatenate([np.eye(64, dtype=np.float32)] * 2, axis=1)  # [64,128]
    return id64, np.ascontiguousarray(id2)


_last_results = {}


def kernel(x: np.ndarray, weight: np.ndarray) -> np.ndarray:
    mode = os.environ.get("BITCONV_MODE", "fused")
    trace = os.environ.get("BITCONV_TRACE", "0") == "1"
    if not trace:
        # The NTFF profile hook is unavailable through this axon client;
        # make sure nothing engages the trace path.
        os.environ.setdefault("BASS_NEVER_TRACE", "1")
    x = np.ascontiguousarray(x, dtype=np.float32)
    w = np.ascontiguousarray(weight.reshape(64, 64), dtype=np.float32)
    id64, id2 = _consts()
    core_ids = list(range(N_CORES))
    xs = x.reshape(N_CORES, P, FREE)  # 2 images x 64 ch on partitions

    if mode == "fused":
        if "fused" not in _cache:
            _cache["fused"] = build_fused()
        nc = _cache["fused"]
        in_maps = [
            {"x": xs[i], "w": w, "id64": id64, "id2": id2} for i in core_ids
        ]
        res = run_bass_kernel_spmd(nc, in_maps, core_ids, trace=trace)
        _last_results["fused"] = res
        y = np.stack([res.results[i]["y"] for i in core_ids])
        return np.ascontiguousarray(y.reshape(NB, C, H, W), dtype=np.float32)

    # twopass (exact)
    if "maxp" not in _cache:
        _cache["maxp"] = build_maxpass()
        _cache["mainp"] = build_mainpass()
    res1 = run_bass_kernel_spmd(
        _cache["maxp"], [{"x": xs[i]} for i in core_ids], core_ids, trace=trace
    )
    _last_results["maxp"] = res1
    maxabs = float(np.max([res1.results[i]["m"] for i in core_ids]))
    m = np.float32(max(min(maxabs, 8.0), EPS))
    s = m / np.float32(QMAX)
    beta = np.float32(1.0) / s
    scales = np.array([[beta, s]], dtype=np.float32)
    in_maps = [
        {"x": xs[i], "w": w, "id64": id64, "id2": id2, "scales": scales}
        for i in core_ids
    ]
    res2 = run_bass_kernel_spmd(_cache["mainp"], in_maps, core_ids, trace=trace)
    _last_results["mainp"] = res2
    y = np.stack([res2.results[i]["y"] for i in core_ids])
    return np.ascontiguousarray(y.reshape(NB, C, H, W), dtype=np.float32)
